# revision 15
# baseline (speedup 1.0000x reference)
"""Trainium2 Bass kernel for a pre-LN multi-head attention block.

Model (per batch b): LayerNorm(x) -> QKV -> 16-head attention (dh=64) ->
output projection + bias.

Sharding over 8 NeuronCores: core c handles batch b = c//2 and head
group g = c%2 (heads 8g..8g+7, all 2048 queries, full 2048 keys).  No
work is duplicated across the pair except the LayerNorm: each core
projects only its own 8 heads' q/k/v and multiplies its 512 inner
features into w_out, producing a partial [2048, 1024] output; the host
sums the two partials per batch (plus b_out).  No collectives.

Device-side layout notes:
 - Activations are kept transposed (feature dim on partitions): every
   matmul contracts over the partition axis.
 - LayerNorm stats run on the vector engine (bn_stats/bn_aggr) and the
   (x-mu)*rstd is one fused tensor_scalar pass, keeping the scalar
   engine free for the softmax exp (the ACT engine is the 2nd-busiest
   resource after the PE array).
 - Scores are computed directly as S^T [nk, nq]; softmax needs no max
   subtraction (scores ~ N(0,1)), so exp is one ScalarE pass and the
   denominator rides along as a ones-column in the PV matmul (M=65).
 - Softmax denominators are reciprocated on the scalar engine ([1,512]
   tiles are single-lane on DVE and cost ~2.3us there vs ~0.6us on ACT).
 - ln_gamma/ln_beta are folded into the QKV weights host-side; b_out is
   added host-side.
 - QKV weights live in SBUF whole (8 tiles of [128,1536] bf16, one DMA
   each); per-pair q/k/v tiles are produced right before that pair's
   attention, interleaved into the previous pair's kt loop so TensorE
   stays densely busy.
 - The output projection for query block qb is emitted right after the
   last head pair's qb normalization, so it overlaps the tail instead
   of serializing after all attention.
"""

import numpy as np
from ml_dtypes import bfloat16

B, N, D = 4, 2048, 1024
HEADS, DH = 16, 64
SCALE = DH ** -0.5
NCORES = 8
HLOC = 8                    # heads per core
INNER_LOC = HLOC * DH       # 512
EPS = 1e-5
NT = N // 128               # 16 sequence tiles (LN)
KD = D // 128               # 8 feature tiles
NKT = N // 128              # 16 key tiles
NPAIR = HLOC // 2           # 4 head pairs per core
NQB = N // 512              # 4 query blocks of 512
NOB = 3 * INNER_LOC // 128  # 12 qkv output row-tiles (q:0-3, k:4-7, v:8-11)
KQ = INNER_LOC // 128       # 4 inner-feature tiles for the out projection

_cache = {}


def _build():
    import concourse.bass as bass
    import concourse.mybir as mybir
    import concourse.bacc as bacc
    import concourse.tile as tile
    from concourse.masks import make_identity

    f32 = mybir.dt.float32
    bf16 = mybir.dt.bfloat16
    AX = mybir.AxisListType
    ALU = mybir.AluOpType
    ACTF = mybir.ActivationFunctionType

    nc = bacc.Bacc(
        "TRN2",
        target_bir_lowering=False,
        debug=False,
        enable_asserts=True,
        num_devices=NCORES,
    )

    x_d = nc.dram_tensor("x", [N, D], f32, kind="ExternalInput").ap()
    wq_d = nc.dram_tensor("wqkvT", [D, 3 * INNER_LOC], bf16,
                          kind="ExternalInput").ap()
    bias_d = nc.dram_tensor("qkv_bias", [128, NOB], f32,
                            kind="ExternalInput").ap()
    wo_d = nc.dram_tensor("woutT", [INNER_LOC, D], bf16,
                          kind="ExternalInput").ap()
    out_d = nc.dram_tensor("out", [N, D], f32, kind="ExternalOutput").ap()

    with tile.TileContext(nc) as tc:
        with (
            tc.tile_pool(name="persist", bufs=1) as P,
            tc.tile_pool(name="ppool", bufs=1, space="PSUM") as PS,
            tc.tile_pool(name="trans", bufs=1) as T,
        ):
            ident = P.tile([128, 128], bf16, name="ident", tag="ident")
            make_identity(nc, ident)
            eps_t = P.tile([128, 1], f32, name="eps_t", tag="eps_t")
            nc.vector.memset(eps_t, EPS)

            bias_sb = P.tile([128, NOB], f32, name="bias_sb", tag="bias_sb")
            nc.sync.dma_start(bias_sb, bias_d)

            # whole QKV weight resident: 8 tiles [128, 1536] bf16
            wq_sb = []
            for k in range(KD):
                t = P.tile([128, 3 * INNER_LOC], bf16, name=f"wq{k}",
                           tag=f"wq{k}")
                nc.sync.dma_start(t, wq_d[k * 128:(k + 1) * 128, :])
                wq_sb.append(t)
            wo_sb = []

            # xnT: transposed normalized activations [d, n] as [128, KD*N]
            xnT = P.tile([128, KD * N], bf16, name="xnT", tag="xnT")
            xnT3 = xnT.rearrange("p (k n) -> p k n", k=KD)
            # normalized attention outputs, transposed: [512 hd, 2048 nq]
            onormT = []
            for p_ in range(KQ):
                onormT.append(
                    P.tile([128, N], bf16, name=f"onormT{p_}", tag=f"onormT{p_}")
                )

            # persistent V_ext ring: [h2][parity] tiles of [nk, 65] blocks;
            # ones column memset once, V part overwritten per pair
            ve_ring = [[None, None], [None, None]]
            for h2 in range(2):
                for par in range(2):
                    ve = P.tile([128, NKT * 65], bf16,
                                name=f"vx{h2}_{par}", tag=f"vx{h2}_{par}")
                    nc.vector.memset(ve, 1.0)
                    ve_ring[h2][par] = ve.rearrange("p (k e) -> p k e", e=65)

            # QKV projection work for pair j is packaged as a list of
            # emission closures so it can be interleaved into pair j-1's
            # attention loop.
            def qkv_emitters(j, store):
                ems = []
                reqs = []
                for key in ("qT", "kT", "vT"):
                    def alloc(j=j, key=key):
                        store[key] = T.tile(
                            [128, N], bf16, name=f"t{key}{j}", tag=key,
                            bufs=3,
                        )
                    ems.append(alloc)
                    reqs.append(-1)
                # 12 (ob, chunk) units in LN-availability order; emit them
                # PAIRWISE with the two 8-matmul accumulation chains
                # interleaved so the PE always has two independent streams.
                units = []
                for c in range(N // 512):
                    for ob, key in ((j, "qT"), (NPAIR + j, "kT"),
                                    (2 * NPAIR + j, "vT")):
                        units.append((ob, c, key))
                for u in range(0, len(units), 2):
                    ua, ub = units[u], units[u + 1]
                    def chunk2(j=j, ua=ua, ub=ub):
                        qps = []
                        for (ob, c, key), nm in ((ua, "a"), (ub, "b")):
                            qps.append(PS.tile(
                                [128, 512], f32, name=f"qp{j}_{nm}_{ob}_{c}",
                                tag="work", bufs=2,
                            ))
                        for k in range(KD):
                            for (ob, c, key), qp in zip((ua, ub), qps):
                                nc.tensor.matmul(
                                    qp,
                                    lhsT=wq_sb[k][:, ob * 128:(ob + 1) * 128],
                                    rhs=xnT3[:, k, c * 512:(c + 1) * 512],
                                    start=(k == 0),
                                    stop=(k == KD - 1),
                                )
                        for (ob, c, key), qp in zip((ua, ub), qps):
                            dcol = store[key][:, c * 512:(c + 1) * 512]
                            nc.vector.tensor_scalar_add(
                                dcol, qp, bias_sb[:, ob:ob + 1]
                            )
                    ems.append(chunk2)
                    reqs.append(4 * max(ua[1], ub[1]) + 3)
                # V_ext for the two heads: [nk, dh | ones] blocks per key tile
                for h2 in range(2):
                    for g2 in range(2):
                        def vtr(j=j, h2=h2, g2=g2):
                            p0 = h2 * 64
                            id64 = ident[p0:p0 + 64, p0:p0 + 64]
                            vT_j = store["vT"]
                            ve3 = ve_ring[h2][j % 2]
                            tp = PS.tile(
                                [128, 512], bf16, name=f"vt{j}_{h2}_{g2}",
                                tag="work", bufs=2,
                            )
                            for i8 in range(8):
                                kt = g2 * 8 + i8
                                nc.tensor.transpose(
                                    tp[:, i8 * 64:(i8 + 1) * 64],
                                    vT_j[p0:p0 + 64, kt * 128:(kt + 1) * 128],
                                    id64,
                                )
                            dest = ve3[:, g2 * 8:(g2 + 1) * 8, 0:64]
                            src = tp.rearrange("p (k e) -> p k e", e=64)
                            nc.vector.tensor_copy(dest, src)
                        ems.append(vtr)
                        reqs.append(8 * g2 + 7)
                return ems, reqs

            stores = [dict() for _ in range(NPAIR)]
            ems0, reqs0 = qkv_emitters(0, stores[0])
            order0 = sorted(range(len(ems0)), key=lambda i: reqs0[i])
            ems0 = [ems0[i] for i in order0]
            reqs0 = [reqs0[i] for i in order0]
            e0i = 0

            # ---- Phase A: LayerNorm + transpose, pipelined over seq tiles.
            # Stats on DVE (bn_stats/bn_aggr), normalize is one fused
            # tensor_scalar pass; ACT only does the tiny rsqrt.
            for nt in range(NT):
                x_t = T.tile([128, D], f32, name=f"x{nt}", tag="x", bufs=3)
                nc.sync.dma_start(x_t, x_d[nt * 128:(nt + 1) * 128, :])
                x3 = x_t.rearrange("p (c f) -> p c f", c=2)
                st6 = T.tile([128, 2, 6], f32, name=f"st{nt}", tag="st", bufs=3)
                nc.vector.bn_stats(st6[:, 0, :], x3[:, 0, :])
                nc.vector.bn_stats(st6[:, 1, :], x3[:, 1, :])
                mv = T.tile([128, 2], f32, name=f"mv{nt}", tag="mv", bufs=3)
                nc.vector.bn_aggr(mv, st6)
                std = T.tile([128, 1], f32, name=f"sd{nt}", tag="sd", bufs=3)
                nc.scalar.activation(std, mv[:, 1:2], ACTF.Sqrt, bias=eps_t)
                rstd = T.tile([128, 1], f32, name=f"rs{nt}", tag="rs", bufs=3)
                nc.vector.reciprocal(rstd, std)
                xhat = T.tile([128, D], bf16, name=f"xh{nt}", tag="xh", bufs=3)
                nc.vector.tensor_scalar(
                    xhat, x_t, mv[:, 0:1], rstd, ALU.subtract, ALU.mult
                )
                for g2 in range(2):
                    tp = PS.tile(
                        [128, 512], bf16, name=f"tp{nt}_{g2}",
                        tag="work", bufs=2
                    )
                    for j in range(4):
                        kd = g2 * 4 + j
                        nc.tensor.transpose(
                            tp[:, j * 128:(j + 1) * 128],
                            xhat[:, kd * 128:(kd + 1) * 128],
                            ident,
                        )
                    dest = xnT3[:, g2 * 4:(g2 + 1) * 4, nt * 128:(nt + 1) * 128]
                    src = tp.rearrange("p (k n) -> p k n", k=4)
                    # ACT is idle during LN (stats run on DVE); GPSIMD can't
                    # read PSUM, so ACT takes all these evictions.
                    nc.scalar.copy(dest, src)
                # interleave pair-0 QKV emission once its LN inputs
                # have been emitted (Tile deps follow emission order)
                while e0i < len(ems0) and reqs0[e0i] <= nt:
                    ems0[e0i]()
                    e0i += 1

            # ---- Phases B+C: per head pair, attention row-packed via
            # tile_position so K stays covered.
            while e0i < len(ems0):
                ems0[e0i]()
                e0i += 1

            def norm_chain(p_, qb, ocs, dens):
                # ocs[h2] [64,512] = O^T; dens[h2] [1,512] = softmax denoms
                b0 = qb * 512
                for h2 in range(2):
                    p0 = h2 * 64
                    rl = T.tile([1, 512], f32, name=f"rl{p_}_{h2}_{qb}",
                                tag="rl", bufs=3)
                    nc.vector.reciprocal(rl, dens[h2])
                    rlb = T.tile([64, 512], f32, name=f"rlb{p_}_{h2}_{qb}",
                                 tag="rlb", bufs=3)
                    nc.gpsimd.partition_broadcast(rlb, rl, channels=64)
                    nc.vector.tensor_mul(
                        onormT[p_][p0:p0 + 64, b0:b0 + 512],
                        ocs[h2],
                        rlb,
                    )

            def proj_qb(qb):
                # out rows qb*512 .. qb*512+512, all 1024 cols; the two
                # 512-col accumulation chains are interleaved (2 PE streams)
                for nt in range(4 * qb, 4 * qb + 4):
                    po = T.tile([128, D], f32, name=f"po{nt}", tag="po",
                                bufs=2)
                    pps = [
                        PS.tile([128, 512], f32, name=f"pp{nt}_{c}",
                                tag="work", bufs=2)
                        for c in range(2)
                    ]
                    for kq in range(KQ):
                        for c in range(2):
                            nc.tensor.matmul(
                                pps[c],
                                lhsT=onormT[kq][:, nt * 128:(nt + 1) * 128],
                                rhs=wo_sb[kq][:, c * 512:(c + 1) * 512],
                                start=(kq == 0),
                                stop=(kq == KQ - 1),
                            )
                    for c in range(2):
                        nc.vector.tensor_copy(po[:, c * 512:(c + 1) * 512], pps[c])
                    nc.sync.dma_start(out_d[nt * 128:(nt + 1) * 128, :], po)

            for j in range(NPAIR):
                st = stores[j]
                if j == 2:
                    for kq in range(KQ):
                        t = P.tile([128, D], bf16, name=f"wo{kq}", tag=f"wo{kq}")
                        nc.sync.dma_start(t, wo_d[kq * 128:(kq + 1) * 128, :])
                        wo_sb.append(t)
                qT_j, kT_j = st["qT"], st["kT"]
                ve3s = [ve_ring[0][j % 2], ve_ring[1][j % 2]]
                if j + 1 < NPAIR:
                    pe_, pr_ = qkv_emitters(j + 1, stores[j + 1])
                    po_ = sorted(range(len(pe_)), key=lambda i: pr_[i])
                    pend = [pe_[i] for i in po_]
                else:
                    pend = []
                pi = 0
                norm_defer = []

                def drain_norms(limit):
                    # run deferred normalize chains for blocks < limit; for
                    # the last pair, follow each with that block's out-proj
                    while norm_defer and norm_defer[0][0] < limit:
                        qb_, ocs_, den_ = norm_defer.pop(0)
                        norm_chain(j, qb_, ocs_, den_)
                        if j == NPAIR - 1:
                            proj_qb(qb_)

                for qb in range(NQB):
                    # previous block's norm first: its inputs are ready and
                    # emitting it ahead keeps it early in the DVE queue, so
                    # the proj matmuls it feeds can fill PE gaps in this block
                    drain_norms(qb)
                    b0 = qb * 512
                    opss = [
                        PS.tile([65, 512], f32, name=f"ops{2*j}_{qb}",
                                tag="acc0", bufs=1),
                        PS.tile([65, 512], f32, name=f"ops{2*j+1}_{qb}",
                                tag="acc1", bufs=1),
                    ]
                    for kt in range(NKT):
                        sps = PS.tile(
                            [128, 1024], f32, name=f"s{j}_{qb}_{kt}",
                            tag="spair", bufs=2,
                        )
                        for h2 in range(2):
                            p0 = h2 * 64
                            nc.tensor.matmul(
                                sps[:, h2 * 512:(h2 + 1) * 512],
                                lhsT=kT_j[p0:p0 + 64, kt * 128:(kt + 1) * 128],
                                rhs=qT_j[p0:p0 + 64, b0:b0 + 512],
                                start=True,
                                stop=True,
                                tile_position=(p0, 0),
                            )
                        pt = T.tile(
                            [128, 1024], bf16, name=f"pt{j}_{qb}_{kt}",
                            tag="pt", bufs=10,
                        )
                        nc.scalar.activation(pt, sps, ACTF.Exp, scale=SCALE)
                        for h2 in range(2):
                            nc.tensor.matmul(
                                opss[h2],
                                lhsT=ve3s[h2][:, kt, :],
                                rhs=pt[:, h2 * 512:(h2 + 1) * 512],
                                start=(kt == 0),
                                stop=(kt == NKT - 1),
                            )
                        # interleave next-pair QKV emission across kt slots
                        it = qb * NKT + kt + 1
                        while pi < len(pend) and pi * NKT * NQB < len(pend) * it:
                            pend[pi]()
                            pi += 1
                    # evict accumulators to SBUF fast (frees the psum bank)
                    ocs = []
                    dens = []
                    for h2 in range(2):
                        oc = T.tile([64, 512], f32, name=f"oc{j}_{h2}_{qb}",
                                    tag="oc", bufs=5)
                        nc.vector.tensor_copy(oc, opss[h2][0:64, :])
                        den = T.tile([1, 512], f32, name=f"den{j}_{h2}_{qb}",
                                     tag="den", bufs=5)
                        nc.vector.tensor_copy(den, opss[h2][64:65, :])
                        ocs.append(oc)
                        dens.append(den)
                    norm_defer.append((qb, ocs, dens))
                while pi < len(pend):
                    pend[pi]()
                    pi += 1
                drain_norms(NQB)

    nc.compile()
    return nc


def _shard_inputs(x, ln_gamma, ln_beta, w_qkv, w_out):
    in_maps = []
    for g in range(2):
        rq = slice(g * INNER_LOC, (g + 1) * INNER_LOC)
        rk = slice(D + g * INNER_LOC, D + (g + 1) * INNER_LOC)
        rv = slice(2 * D + g * INNER_LOC, 2 * D + (g + 1) * INNER_LOC)
        w_slice = np.concatenate(
            [w_qkv[rq], w_qkv[rk], w_qkv[rv]], axis=0
        )                                                           # [1536, 1024]
        w_eff = w_slice * ln_gamma[None, :]
        wqkvT = np.ascontiguousarray(w_eff.T).astype(bfloat16)      # [1024, 1536]
        bias = w_slice.astype(np.float64) @ ln_beta.astype(np.float64)
        bias_2d = np.ascontiguousarray(
            bias.reshape(NOB, 128).T
        ).astype(np.float32)                                        # [128, 12]
        woutT = np.ascontiguousarray(
            w_out[:, g * INNER_LOC:(g + 1) * INNER_LOC].T
        ).astype(bfloat16)                                          # [512, 1024]
        in_maps.append((wqkvT, bias_2d, woutT))

    full = []
    for c in range(NCORES):
        b, g = c // 2, c % 2
        wqkvT, bias_2d, woutT = in_maps[g]
        full.append({
            "x": np.ascontiguousarray(np.asarray(x[b], dtype=np.float32)),
            "wqkvT": wqkvT,
            "qkv_bias": bias_2d,
            "woutT": woutT,
        })
    return full


def kernel(x, ln_gamma, ln_beta, w_qkv, w_out, b_out, _trace=False):
    from concourse import bass_utils

    x = np.asarray(x, dtype=np.float32)
    ln_gamma = np.asarray(ln_gamma, dtype=np.float32)
    ln_beta = np.asarray(ln_beta, dtype=np.float32)
    w_qkv = np.asarray(w_qkv, dtype=np.float32)
    w_out = np.asarray(w_out, dtype=np.float32)
    b_out = np.asarray(b_out, dtype=np.float32)

    if "nc" not in _cache:
        _cache["nc"] = _build()
    nc = _cache["nc"]

    in_maps = _shard_inputs(x, ln_gamma, ln_beta, w_qkv, w_out)
    res = bass_utils.run_bass_kernel_spmd(
        nc, in_maps, core_ids=list(range(NCORES)), trace=_trace
    )
    out = np.empty((B, N, D), dtype=np.float32)
    for b in range(B):
        out[b] = np.asarray(res.results[2 * b]["out"])
        out[b] += np.asarray(res.results[2 * b + 1]["out"])
    out += b_out[None, None, :]
    _cache["last_result"] = res
    return out


# revision 20
# speedup vs baseline: 1.1006x; 1.1006x over previous
"""Trainium2 Bass kernel for a pre-LN multi-head attention block.

Model (per batch b): LayerNorm(x) -> QKV -> 16-head attention (dh=64) ->
output projection + bias.

Sharding over 8 NeuronCores: core c handles batch b = c//2 and head
group g = c%2 (heads 8g..8g+7, all 2048 queries, full 2048 keys).  No
work is duplicated across the pair except the LayerNorm: each core
projects only its own 8 heads' q/k/v and multiplies its 512 inner
features into w_out, producing a partial [2048, 1024] output; the host
sums the two partials per batch (plus b_out).  No collectives.

Device-side layout notes:
 - Activations are kept transposed (feature dim on partitions): every
   matmul contracts over the partition axis.
 - LayerNorm stats run on the vector engine (bn_stats/bn_aggr) and the
   (x-mu)*rstd is one fused tensor_scalar pass, keeping the scalar
   engine free for the softmax exp (the ACT engine is the 2nd-busiest
   resource after the PE array).
 - Scores are computed directly as S^T [nk, nq]; softmax needs no max
   subtraction (scores ~ N(0,1)), so exp is one ScalarE pass and the
   denominator rides along as a ones-column in the PV matmul (M=65).
 - Softmax denominators are reciprocated on the scalar engine ([1,512]
   tiles are single-lane on DVE and cost ~2.3us there vs ~0.6us on ACT).
 - ln_gamma/ln_beta are folded into the QKV weights host-side; b_out is
   added host-side.
 - QKV weights live in SBUF whole (8 tiles of [128,1536] bf16, one DMA
   each); per-pair q/k/v tiles are produced right before that pair's
   attention, interleaved into the previous pair's kt loop so TensorE
   stays densely busy.
 - The output projection for query block qb is emitted right after the
   last head pair's qb normalization, so it overlaps the tail instead
   of serializing after all attention.
"""

import numpy as np
from ml_dtypes import bfloat16

B, N, D = 4, 2048, 1024
HEADS, DH = 16, 64
SCALE = DH ** -0.5
NCORES = 8
HLOC = 8                    # heads per core
INNER_LOC = HLOC * DH       # 512
EPS = 1e-5
NT = N // 128               # 16 sequence tiles (LN)
KD = D // 128               # 8 feature tiles
NKT = N // 128              # 16 key tiles
NPAIR = HLOC // 2           # 4 head pairs per core
NQB = N // 512              # 4 query blocks of 512
NOB = 3 * INNER_LOC // 128  # 12 qkv output row-tiles (q:0-3, k:4-7, v:8-11)
KQ = INNER_LOC // 128       # 4 inner-feature tiles for the out projection

_cache = {}


def _build():
    import concourse.bass as bass
    import concourse.mybir as mybir
    import concourse.bacc as bacc
    import concourse.tile as tile
    from concourse.masks import make_identity

    f32 = mybir.dt.float32
    bf16 = mybir.dt.bfloat16
    AX = mybir.AxisListType
    ALU = mybir.AluOpType
    ACTF = mybir.ActivationFunctionType

    nc = bacc.Bacc(
        "TRN2",
        target_bir_lowering=False,
        debug=False,
        enable_asserts=True,
        num_devices=NCORES,
    )

    x_d = nc.dram_tensor("x", [N, D], f32, kind="ExternalInput").ap()
    wq_d = nc.dram_tensor("wqkvT", [D, 3 * INNER_LOC], bf16,
                          kind="ExternalInput").ap()
    bias_d = nc.dram_tensor("qkv_bias", [128, NOB], f32,
                            kind="ExternalInput").ap()
    wo_d = nc.dram_tensor("woutT", [INNER_LOC, D], bf16,
                          kind="ExternalInput").ap()
    out_d = nc.dram_tensor("out", [N, D], f32, kind="ExternalOutput").ap()

    with tile.TileContext(nc) as tc:
        with (
            tc.tile_pool(name="persist", bufs=1) as P,
            tc.tile_pool(name="ppool", bufs=1, space="PSUM") as PS,
            tc.tile_pool(name="trans", bufs=1) as T,
        ):
            ident = P.tile([128, 128], bf16, name="ident", tag="ident")
            make_identity(nc, ident)
            eps_t = P.tile([128, 1], f32, name="eps_t", tag="eps_t")
            nc.vector.memset(eps_t, EPS)

            bias_sb = P.tile([128, NOB], f32, name="bias_sb", tag="bias_sb")
            nc.sync.dma_start(bias_sb, bias_d)

            # whole QKV weight resident: 8 tiles [128, 1536] bf16
            wq_sb = []
            for k in range(KD):
                t = P.tile([128, 3 * INNER_LOC], bf16, name=f"wq{k}",
                           tag=f"wq{k}")
                nc.sync.dma_start(t, wq_d[k * 128:(k + 1) * 128, :])
                wq_sb.append(t)
            wo_sb = []

            # xnT: transposed normalized activations [d, n] as [128, KD*N]
            xnT = P.tile([128, KD * N], bf16, name="xnT", tag="xnT")
            xnT3 = xnT.rearrange("p (k n) -> p k n", k=KD)
            # normalized attention outputs, transposed: [512 hd, 2048 nq]
            onormT = []
            for p_ in range(KQ):
                onormT.append(
                    P.tile([128, N], bf16, name=f"onormT{p_}", tag=f"onormT{p_}")
                )

            # persistent V_ext ring: [h2][parity] tiles of [nk, 65] blocks;
            # ones column memset once, V part overwritten per pair
            ve_ring = [[None, None], [None, None]]
            for h2 in range(2):
                for par in range(2):
                    ve = P.tile([128, NKT * 65], bf16,
                                name=f"vx{h2}_{par}", tag=f"vx{h2}_{par}")
                    nc.vector.memset(ve, 1.0)
                    ve_ring[h2][par] = ve.rearrange("p (k e) -> p k e", e=65)

            # QKV projection work for pair j is packaged as a list of
            # emission closures so it can be interleaved into pair j-1's
            # attention loop.
            def qkv_emitters(j, store):
                ems = []
                reqs = []
                for key in ("qT", "kT", "vT"):
                    def alloc(j=j, key=key):
                        store[key] = T.tile(
                            [128, N], bf16, name=f"t{key}{j}", tag=key,
                            bufs=3,
                        )
                    ems.append(alloc)
                    reqs.append(-1)
                for c in range(N // 512):
                    for ob, key in ((j, "qT"), (NPAIR + j, "kT"),
                                    (2 * NPAIR + j, "vT")):
                        def chunk(j=j, c=c, ob=ob, key=key):
                            qp = PS.tile(
                                [128, 512], f32, name=f"qp{key}{j}_{c}",
                                tag="work", bufs=2,
                            )
                            for k in range(KD):
                                nc.tensor.matmul(
                                    qp,
                                    lhsT=wq_sb[k][:, ob * 128:(ob + 1) * 128],
                                    rhs=xnT3[:, k, c * 512:(c + 1) * 512],
                                    start=(k == 0),
                                    stop=(k == KD - 1),
                                )
                            dcol = store[key][:, c * 512:(c + 1) * 512]
                            nc.vector.tensor_scalar_add(
                                dcol, qp, bias_sb[:, ob:ob + 1]
                            )
                        ems.append(chunk)
                        reqs.append(4 * c + 3)
                # V_ext for the two heads: [nk, dh | ones] blocks per key tile
                for h2 in range(2):
                    for g2 in range(2):
                        def vtr(j=j, h2=h2, g2=g2):
                            p0 = h2 * 64
                            id64 = ident[p0:p0 + 64, p0:p0 + 64]
                            vT_j = store["vT"]
                            ve3 = ve_ring[h2][j % 2]
                            tp = PS.tile(
                                [128, 512], bf16, name=f"vt{j}_{h2}_{g2}",
                                tag="work", bufs=2,
                            )
                            for i8 in range(8):
                                kt = g2 * 8 + i8
                                nc.tensor.transpose(
                                    tp[:, i8 * 64:(i8 + 1) * 64],
                                    vT_j[p0:p0 + 64, kt * 128:(kt + 1) * 128],
                                    id64,
                                )
                            dest = ve3[:, g2 * 8:(g2 + 1) * 8, 0:64]
                            src = tp.rearrange("p (k e) -> p k e", e=64)
                            nc.vector.tensor_copy(dest, src)
                        ems.append(vtr)
                        reqs.append(8 * g2 + 7)
                return ems, reqs

            stores = [dict() for _ in range(NPAIR)]
            ems0, reqs0 = qkv_emitters(0, stores[0])
            order0 = sorted(range(len(ems0)), key=lambda i: reqs0[i])
            ems0 = [ems0[i] for i in order0]
            reqs0 = [reqs0[i] for i in order0]
            e0i = 0

            # ---- Phase A: LayerNorm + transpose, pipelined over seq tiles.
            # Stats on DVE (bn_stats/bn_aggr), normalize is one fused
            # tensor_scalar pass; ACT only does the tiny rsqrt.
            for nt in range(NT):
                x_t = T.tile([128, D], f32, name=f"x{nt}", tag="x", bufs=3)
                nc.sync.dma_start(x_t, x_d[nt * 128:(nt + 1) * 128, :])
                x3 = x_t.rearrange("p (c f) -> p c f", c=2)
                st6 = T.tile([128, 2, 6], f32, name=f"st{nt}", tag="st", bufs=3)
                nc.vector.bn_stats(st6[:, 0, :], x3[:, 0, :])
                nc.vector.bn_stats(st6[:, 1, :], x3[:, 1, :])
                mv = T.tile([128, 2], f32, name=f"mv{nt}", tag="mv", bufs=3)
                nc.vector.bn_aggr(mv, st6)
                std = T.tile([128, 1], f32, name=f"sd{nt}", tag="sd", bufs=3)
                nc.scalar.activation(std, mv[:, 1:2], ACTF.Sqrt, bias=eps_t)
                rstd = T.tile([128, 1], f32, name=f"rs{nt}", tag="rs", bufs=3)
                nc.vector.reciprocal(rstd, std)
                xhat = T.tile([128, D], bf16, name=f"xh{nt}", tag="xh", bufs=3)
                nc.vector.tensor_scalar(
                    xhat, x_t, mv[:, 0:1], rstd, ALU.subtract, ALU.mult
                )
                for g2 in range(2):
                    tp = PS.tile(
                        [128, 512], bf16, name=f"tp{nt}_{g2}",
                        tag="work", bufs=2
                    )
                    for j in range(4):
                        kd = g2 * 4 + j
                        nc.tensor.transpose(
                            tp[:, j * 128:(j + 1) * 128],
                            xhat[:, kd * 128:(kd + 1) * 128],
                            ident,
                        )
                    dest = xnT3[:, g2 * 4:(g2 + 1) * 4, nt * 128:(nt + 1) * 128]
                    src = tp.rearrange("p (k n) -> p k n", k=4)
                    # ACT is idle during LN (stats run on DVE); GPSIMD can't
                    # read PSUM, so ACT takes all these evictions.
                    nc.scalar.copy(dest, src)
                # interleave pair-0 QKV emission once its LN inputs
                # have been emitted (Tile deps follow emission order)
                while e0i < len(ems0) and reqs0[e0i] <= nt:
                    ems0[e0i]()
                    e0i += 1

            # ---- Phases B+C: per head pair, attention row-packed via
            # tile_position so K stays covered.
            while e0i < len(ems0):
                ems0[e0i]()
                e0i += 1

            def norm_recip(p_, h2, qb, dens, rlbs):
                # one 3.3us DVE reciprocal; emitted mid-kt-loop so it never
                # head-of-line-blocks the QKV psum evictions on DVE
                rl = T.tile([1, 512], f32, name=f"rl{p_}_{h2}_{qb}",
                            tag="rl", bufs=3)
                nc.vector.reciprocal(rl, dens[h2])
                rlb = T.tile([64, 512], f32, name=f"rlb{p_}_{h2}_{qb}",
                             tag="rlb", bufs=3)
                nc.gpsimd.partition_broadcast(rlb, rl, channels=64)
                rlbs.append(rlb)

            def norm_muls(p_, qb, ocs, rlbs):
                b0 = qb * 512
                for h2 in range(2):
                    p0 = h2 * 64
                    nc.vector.tensor_mul(
                        onormT[p_][p0:p0 + 64, b0:b0 + 512],
                        ocs[h2],
                        rlbs[h2],
                    )

            def proj_qb(qb):
                # out rows qb*512 .. qb*512+512, all 1024 cols; the two
                # 512-col accumulation chains are interleaved (2 PE streams)
                for nt in range(4 * qb, 4 * qb + 4):
                    po = T.tile([128, D], f32, name=f"po{nt}", tag="po",
                                bufs=2)
                    pps = [
                        PS.tile([128, 512], f32, name=f"pp{nt}_{c}",
                                tag="work", bufs=2)
                        for c in range(2)
                    ]
                    for kq in range(KQ):
                        for c in range(2):
                            nc.tensor.matmul(
                                pps[c],
                                lhsT=onormT[kq][:, nt * 128:(nt + 1) * 128],
                                rhs=wo_sb[kq][:, c * 512:(c + 1) * 512],
                                start=(kq == 0),
                                stop=(kq == KQ - 1),
                            )
                    for c in range(2):
                        nc.vector.tensor_copy(po[:, c * 512:(c + 1) * 512], pps[c])
                    nc.sync.dma_start(out_d[nt * 128:(nt + 1) * 128, :], po)

            for j in range(NPAIR):
                st = stores[j]
                if j == 2:
                    for kq in range(KQ):
                        t = P.tile([128, D], bf16, name=f"wo{kq}", tag=f"wo{kq}")
                        nc.sync.dma_start(t, wo_d[kq * 128:(kq + 1) * 128, :])
                        wo_sb.append(t)
                qT_j, kT_j = st["qT"], st["kT"]
                ve3s = [ve_ring[0][j % 2], ve_ring[1][j % 2]]
                if j + 1 < NPAIR:
                    pe_, pr_ = qkv_emitters(j + 1, stores[j + 1])
                    po_ = sorted(range(len(pe_)), key=lambda i: pr_[i])
                    pend = [pe_[i] for i in po_]
                else:
                    pend = []
                pi = 0
                norm_defer = []
                for qb in range(NQB):
                    b0 = qb * 512
                    opss = [
                        PS.tile([65, 512], f32, name=f"ops{2*j}_{qb}",
                                tag="acc0", bufs=1),
                        PS.tile([65, 512], f32, name=f"ops{2*j+1}_{qb}",
                                tag="acc1", bufs=1),
                    ]
                    for kt in range(NKT):
                        sps = PS.tile(
                            [128, 1024], f32, name=f"s{j}_{qb}_{kt}",
                            tag="spair", bufs=2,
                        )
                        for h2 in range(2):
                            p0 = h2 * 64
                            nc.tensor.matmul(
                                sps[:, h2 * 512:(h2 + 1) * 512],
                                lhsT=kT_j[p0:p0 + 64, kt * 128:(kt + 1) * 128],
                                rhs=qT_j[p0:p0 + 64, b0:b0 + 512],
                                start=True,
                                stop=True,
                                tile_position=(p0, 0),
                            )
                        pt = T.tile(
                            [128, 1024], bf16, name=f"pt{j}_{qb}_{kt}",
                            tag="pt", bufs=10,
                        )
                        nc.scalar.activation(pt, sps, ACTF.Exp, scale=SCALE)
                        for h2 in range(2):
                            nc.tensor.matmul(
                                opss[h2],
                                lhsT=ve3s[h2][:, kt, :],
                                rhs=pt[:, h2 * 512:(h2 + 1) * 512],
                                start=(kt == 0),
                                stop=(kt == NKT - 1),
                            )
                        # interleave next-pair QKV emission across kt slots
                        it = qb * NKT + kt + 1
                        while pi < len(pend) and pi * NKT * NQB < len(pend) * it:
                            pend[pi]()
                            pi += 1
                        # previous block's normalize chain, spread across
                        # this block's kt slots: the 3.3us reciprocals land
                        # between QKV psum evictions in the DVE queue instead
                        # of in front of them, and the pair-3 out-projection
                        # fills PE slack while this block's exps stream
                        if norm_defer:
                            qb_, ocs_, dens_, rlbs_ = norm_defer[0]
                            if kt == 4:
                                norm_recip(j, 0, qb_, dens_, rlbs_)
                            elif kt == 6:
                                norm_recip(j, 1, qb_, dens_, rlbs_)
                            elif kt == 9:
                                norm_muls(j, qb_, ocs_, rlbs_)
                            elif kt == 10:
                                norm_defer.pop(0)
                                if j == NPAIR - 1:
                                    proj_qb(qb_)
                    # evict accumulators to SBUF fast (frees the psum bank)
                    ocs = []
                    dens = []
                    for h2 in range(2):
                        oc = T.tile([64, 512], f32, name=f"oc{j}_{h2}_{qb}",
                                    tag="oc", bufs=5)
                        nc.vector.tensor_copy(oc, opss[h2][0:64, :])
                        den = T.tile([1, 512], f32, name=f"den{j}_{h2}_{qb}",
                                     tag="den", bufs=5)
                        nc.vector.tensor_copy(den, opss[h2][64:65, :])
                        ocs.append(oc)
                        dens.append(den)
                    norm_defer.append((qb, ocs, dens, []))
                while pi < len(pend):
                    pend[pi]()
                    pi += 1
                # leftover norm chains (the final block, plus any earlier
                # block still mid-chain)
                while norm_defer:
                    qb_, ocs_, dens_, rlbs_ = norm_defer.pop(0)
                    if len(rlbs_) < 1:
                        norm_recip(j, 0, qb_, dens_, rlbs_)
                    if len(rlbs_) < 2:
                        norm_recip(j, 1, qb_, dens_, rlbs_)
                    norm_muls(j, qb_, ocs_, rlbs_)
                    if j == NPAIR - 1:
                        proj_qb(qb_)

    nc.compile()
    return nc


def _shard_inputs(x, ln_gamma, ln_beta, w_qkv, w_out):
    in_maps = []
    for g in range(2):
        rq = slice(g * INNER_LOC, (g + 1) * INNER_LOC)
        rk = slice(D + g * INNER_LOC, D + (g + 1) * INNER_LOC)
        rv = slice(2 * D + g * INNER_LOC, 2 * D + (g + 1) * INNER_LOC)
        w_slice = np.concatenate(
            [w_qkv[rq], w_qkv[rk], w_qkv[rv]], axis=0
        )                                                           # [1536, 1024]
        w_eff = w_slice * ln_gamma[None, :]
        wqkvT = np.ascontiguousarray(w_eff.T).astype(bfloat16)      # [1024, 1536]
        bias = w_slice.astype(np.float64) @ ln_beta.astype(np.float64)
        bias_2d = np.ascontiguousarray(
            bias.reshape(NOB, 128).T
        ).astype(np.float32)                                        # [128, 12]
        woutT = np.ascontiguousarray(
            w_out[:, g * INNER_LOC:(g + 1) * INNER_LOC].T
        ).astype(bfloat16)                                          # [512, 1024]
        in_maps.append((wqkvT, bias_2d, woutT))

    full = []
    for c in range(NCORES):
        b, g = c // 2, c % 2
        wqkvT, bias_2d, woutT = in_maps[g]
        full.append({
            "x": np.ascontiguousarray(np.asarray(x[b], dtype=np.float32)),
            "wqkvT": wqkvT,
            "qkv_bias": bias_2d,
            "woutT": woutT,
        })
    return full


def kernel(x, ln_gamma, ln_beta, w_qkv, w_out, b_out, _trace=False):
    from concourse import bass_utils

    x = np.asarray(x, dtype=np.float32)
    ln_gamma = np.asarray(ln_gamma, dtype=np.float32)
    ln_beta = np.asarray(ln_beta, dtype=np.float32)
    w_qkv = np.asarray(w_qkv, dtype=np.float32)
    w_out = np.asarray(w_out, dtype=np.float32)
    b_out = np.asarray(b_out, dtype=np.float32)

    if "nc" not in _cache:
        _cache["nc"] = _build()
    nc = _cache["nc"]

    in_maps = _shard_inputs(x, ln_gamma, ln_beta, w_qkv, w_out)
    res = bass_utils.run_bass_kernel_spmd(
        nc, in_maps, core_ids=list(range(NCORES)), trace=_trace
    )
    out = np.empty((B, N, D), dtype=np.float32)
    for b in range(B):
        out[b] = np.asarray(res.results[2 * b]["out"])
        out[b] += np.asarray(res.results[2 * b + 1]["out"])
    out += b_out[None, None, :]
    _cache["last_result"] = res
    return out


# revision 26
# speedup vs baseline: 1.1323x; 1.0288x over previous
"""Trainium2 Bass kernel for a pre-LN multi-head attention block.

Model (per batch b): LayerNorm(x) -> QKV -> 16-head attention (dh=64) ->
output projection + bias.

Sharding over 8 NeuronCores: core c handles batch b = c//2 and head
group g = c%2 (heads 8g..8g+7, all 2048 queries, full 2048 keys).  No
work is duplicated across the pair except the LayerNorm: each core
projects only its own 8 heads' q/k/v and multiplies its 512 inner
features into w_out, producing a partial [2048, 1024] output; the host
sums the two partials per batch (plus b_out).  No collectives.

Device-side layout notes:
 - Activations are kept transposed (feature dim on partitions): every
   matmul contracts over the partition axis.
 - LayerNorm stats run on the vector engine (bn_stats/bn_aggr) and the
   (x-mu)*rstd is one fused tensor_scalar pass, keeping the scalar
   engine free for the softmax exp (the ACT engine is the 2nd-busiest
   resource after the PE array).
 - Scores are computed directly as S^T [nk, nq]; softmax needs no max
   subtraction (scores ~ N(0,1)), so exp is one ScalarE pass and the
   denominator rides along as a ones-column in the PV matmul (M=65).
 - Softmax denominators are reciprocated on the scalar engine ([1,512]
   tiles are single-lane on DVE and cost ~2.3us there vs ~0.6us on ACT).
 - ln_gamma/ln_beta are folded into the QKV weights host-side; b_out is
   added host-side.
 - QKV weights live in SBUF whole (8 tiles of [128,1536] bf16, one DMA
   each); per-pair q/k/v tiles are produced right before that pair's
   attention, interleaved into the previous pair's kt loop so TensorE
   stays densely busy.
 - The output projection for query block qb is emitted right after the
   last head pair's qb normalization, so it overlaps the tail instead
   of serializing after all attention.
"""

import numpy as np
from ml_dtypes import bfloat16

B, N, D = 4, 2048, 1024
HEADS, DH = 16, 64
SCALE = DH ** -0.5
NCORES = 8
HLOC = 8                    # heads per core
INNER_LOC = HLOC * DH       # 512
EPS = 1e-5
NT = N // 128               # 16 sequence tiles (LN)
KD = D // 128               # 8 feature tiles
NKT = N // 128              # 16 key tiles
NPAIR = HLOC // 2           # 4 head pairs per core
NQB = N // 512              # 4 query blocks of 512
NOB = 3 * INNER_LOC // 128  # 12 qkv output row-tiles (q:0-3, k:4-7, v:8-11)
KQ = INNER_LOC // 128       # 4 inner-feature tiles for the out projection

_cache = {}


def _build():
    import concourse.bass as bass
    import concourse.mybir as mybir
    import concourse.bacc as bacc
    import concourse.tile as tile
    from concourse.masks import make_identity

    f32 = mybir.dt.float32
    bf16 = mybir.dt.bfloat16
    AX = mybir.AxisListType
    ALU = mybir.AluOpType
    ACTF = mybir.ActivationFunctionType

    nc = bacc.Bacc(
        "TRN2",
        target_bir_lowering=False,
        debug=False,
        enable_asserts=True,
        num_devices=NCORES,
    )

    x_d = nc.dram_tensor("x", [N, D], f32, kind="ExternalInput").ap()
    wq_d = nc.dram_tensor("wqkvT", [D, 3 * INNER_LOC], bf16,
                          kind="ExternalInput").ap()
    bias_d = nc.dram_tensor("qkv_bias", [128, NOB], f32,
                            kind="ExternalInput").ap()
    wo_d = nc.dram_tensor("woutT", [INNER_LOC, D], bf16,
                          kind="ExternalInput").ap()
    out_d = nc.dram_tensor("out", [N, D], f32, kind="ExternalOutput").ap()

    with tile.TileContext(nc) as tc:
        with (
            tc.tile_pool(name="persist", bufs=1) as P,
            tc.tile_pool(name="ppool", bufs=1, space="PSUM") as PS,
            tc.tile_pool(name="trans", bufs=1) as T,
        ):
            ident = P.tile([128, 128], bf16, name="ident", tag="ident")
            make_identity(nc, ident)
            eps_t = P.tile([128, 1], f32, name="eps_t", tag="eps_t")
            nc.vector.memset(eps_t, EPS)

            bias_sb = P.tile([128, NOB], f32, name="bias_sb", tag="bias_sb")
            nc.sync.dma_start(bias_sb, bias_d)

            # whole QKV weight resident: 8 tiles [128, 1536] bf16
            wq_sb = []
            for k in range(KD):
                t = P.tile([128, 3 * INNER_LOC], bf16, name=f"wq{k}",
                           tag=f"wq{k}")
                nc.sync.dma_start(t, wq_d[k * 128:(k + 1) * 128, :])
                wq_sb.append(t)
            wo_sb = []

            # xnT: transposed normalized activations [d, n] as [128, KD*N]
            xnT = P.tile([128, KD * N], bf16, name="xnT", tag="xnT")
            xnT3 = xnT.rearrange("p (k n) -> p k n", k=KD)
            # normalized attention outputs, transposed: [512 hd, 2048 nq]
            onormT = []
            for p_ in range(KQ):
                onormT.append(
                    P.tile([128, N], bf16, name=f"onormT{p_}", tag=f"onormT{p_}")
                )

            # persistent V_ext ring: [h2][parity] tiles of [nk, 65] blocks;
            # ones column memset once, V part overwritten per pair
            ve_ring = [[None, None], [None, None]]
            for h2 in range(2):
                for par in range(2):
                    ve = P.tile([128, NKT * 65], bf16,
                                name=f"vx{h2}_{par}", tag=f"vx{h2}_{par}")
                    nc.vector.memset(ve, 1.0)
                    ve_ring[h2][par] = ve.rearrange("p (k e) -> p k e", e=65)

            # QKV projection work for pair j is packaged as a list of
            # emission closures so it can be interleaved into pair j-1's
            # attention loop.
            def qkv_emitters(j, store):
                ems = []
                reqs = []
                for key in ("qT", "kT", "vT"):
                    def alloc(j=j, key=key):
                        store[key] = T.tile(
                            [128, N], bf16, name=f"t{key}{j}", tag=key,
                            bufs=3,
                        )
                    ems.append(alloc)
                    reqs.append(-1)
                for c in range(N // 512):
                    for ob, key in ((j, "qT"), (NPAIR + j, "kT"),
                                    (2 * NPAIR + j, "vT")):
                        def chunk(j=j, c=c, ob=ob, key=key):
                            qp = PS.tile(
                                [128, 512], f32, name=f"qp{key}{j}_{c}",
                                tag="work", bufs=2,
                            )
                            for k in range(KD):
                                nc.tensor.matmul(
                                    qp,
                                    lhsT=wq_sb[k][:, ob * 128:(ob + 1) * 128],
                                    rhs=xnT3[:, k, c * 512:(c + 1) * 512],
                                    start=(k == 0),
                                    stop=(k == KD - 1),
                                )
                            dcol = store[key][:, c * 512:(c + 1) * 512]
                            nc.vector.tensor_scalar_add(
                                dcol, qp, bias_sb[:, ob:ob + 1]
                            )
                        ems.append(chunk)
                        reqs.append(4 * c + 3)
                # V_ext for the two heads: [nk, dh | ones] blocks per key tile
                for h2 in range(2):
                    for g2 in range(2):
                        def vtr(j=j, h2=h2, g2=g2):
                            p0 = h2 * 64
                            id64 = ident[p0:p0 + 64, p0:p0 + 64]
                            vT_j = store["vT"]
                            ve3 = ve_ring[h2][j % 2]
                            tp = PS.tile(
                                [128, 512], bf16, name=f"vt{j}_{h2}_{g2}",
                                tag="work", bufs=2,
                            )
                            for i8 in range(8):
                                kt = g2 * 8 + i8
                                nc.tensor.transpose(
                                    tp[:, i8 * 64:(i8 + 1) * 64],
                                    vT_j[p0:p0 + 64, kt * 128:(kt + 1) * 128],
                                    id64,
                                )
                            dest = ve3[:, g2 * 8:(g2 + 1) * 8, 0:64]
                            src = tp.rearrange("p (k e) -> p k e", e=64)
                            nc.vector.tensor_copy(dest, src)
                        ems.append(vtr)
                        reqs.append(8 * g2 + 7)
                return ems, reqs

            stores = [dict() for _ in range(NPAIR)]
            ems0, reqs0 = qkv_emitters(0, stores[0])
            order0 = sorted(range(len(ems0)), key=lambda i: reqs0[i])
            ems0 = [ems0[i] for i in order0]
            reqs0 = [reqs0[i] for i in order0]
            e0i = 0

            # ---- Phase A: LayerNorm + transpose, pipelined over seq tiles.
            # Stats on DVE (bn_stats/bn_aggr), normalize is one fused
            # tensor_scalar pass; ACT only does the tiny rsqrt.
            for nt in range(NT):
                x_t = T.tile([128, D], f32, name=f"x{nt}", tag="x", bufs=3)
                nc.sync.dma_start(x_t, x_d[nt * 128:(nt + 1) * 128, :])
                x3 = x_t.rearrange("p (c f) -> p c f", c=2)
                st6 = T.tile([128, 2, 6], f32, name=f"st{nt}", tag="st", bufs=3)
                nc.vector.bn_stats(st6[:, 0, :], x3[:, 0, :])
                nc.vector.bn_stats(st6[:, 1, :], x3[:, 1, :])
                mv = T.tile([128, 2], f32, name=f"mv{nt}", tag="mv", bufs=3)
                nc.vector.bn_aggr(mv, st6)
                std = T.tile([128, 1], f32, name=f"sd{nt}", tag="sd", bufs=3)
                nc.scalar.activation(std, mv[:, 1:2], ACTF.Sqrt, bias=eps_t)
                rstd = T.tile([128, 1], f32, name=f"rs{nt}", tag="rs", bufs=3)
                nc.vector.reciprocal(rstd, std)
                # normalize on ACT as Copy(rstd*x + (-mu*rstd)): DVE is the
                # phase-A pacer, ACT has slack
                nmr = T.tile([128, 1], f32, name=f"nm{nt}", tag="nm", bufs=3)
                nc.vector.tensor_scalar(
                    nmr, mv[:, 0:1], rstd, -1.0, ALU.mult, ALU.mult
                )
                xhat = T.tile([128, D], bf16, name=f"xh{nt}", tag="xh", bufs=3)
                nc.scalar.activation(xhat, x_t, ACTF.Identity, bias=nmr,
                                     scale=rstd)
                for g2 in range(2):
                    tp = PS.tile(
                        [128, 512], bf16, name=f"tp{nt}_{g2}",
                        tag="work", bufs=2
                    )
                    for j in range(4):
                        kd = g2 * 4 + j
                        nc.tensor.transpose(
                            tp[:, j * 128:(j + 1) * 128],
                            xhat[:, kd * 128:(kd + 1) * 128],
                            ident,
                        )
                    dest = xnT3[:, g2 * 4:(g2 + 1) * 4, nt * 128:(nt + 1) * 128]
                    src = tp.rearrange("p (k n) -> p k n", k=4)
                    # alternate the psum evictions between the two engines
                    # that can read PSUM, to balance phase-A load
                    if (nt + g2) % 2 == 0:
                        nc.vector.tensor_copy(dest, src)
                    else:
                        nc.scalar.copy(dest, src)
                # interleave pair-0 QKV emission once its LN inputs
                # have been emitted (Tile deps follow emission order)
                while e0i < len(ems0) and reqs0[e0i] <= nt:
                    ems0[e0i]()
                    e0i += 1

            # ---- Phases B+C: per head pair, attention row-packed via
            # tile_position so K stays covered.
            while e0i < len(ems0):
                ems0[e0i]()
                e0i += 1

            def norm_recip(p_, h2, qb, dens, rlbs):
                # both heads' denominators are stacked at partitions 0/64 of
                # one tile, so h2==0 pays a single 3.3us DVE reciprocal for
                # both (reciprocal cost is free-size driven).  h2==1 only
                # rehomes its row to partition 0 — partition_broadcast
                # silently reads partition 0 of its source (HW-probed).
                den, rls = dens
                if h2 == 0:
                    rl = T.tile([65, 512], f32, name=f"rl{p_}_{qb}",
                                tag="rl", bufs=3)
                    nc.vector.reciprocal(rl, den)
                    rls.append(rl)
                    src = rl[0:1, :]
                else:
                    rl1 = T.tile([1, 512], f32, name=f"rl1{p_}_{qb}",
                                 tag="rl1", bufs=3)
                    nc.vector.tensor_copy(rl1, rls[0][64:65, :])
                    src = rl1
                rlb = T.tile([64, 512], f32, name=f"rlb{p_}_{h2}_{qb}",
                             tag="rlb", bufs=3)
                nc.gpsimd.partition_broadcast(rlb, src, channels=64)
                rlbs.append(rlb)

            def norm_muls(p_, qb, ocs, rlbs):
                b0 = qb * 512
                for h2 in range(2):
                    p0 = h2 * 64
                    nc.vector.tensor_mul(
                        onormT[p_][p0:p0 + 64, b0:b0 + 512],
                        ocs[h2],
                        rlbs[h2],
                    )

            def proj_qb(qb):
                # out rows qb*512 .. qb*512+512, all 1024 cols; the two
                # 512-col accumulation chains are interleaved (2 PE streams)
                for nt in range(4 * qb, 4 * qb + 4):
                    po = T.tile([128, D], f32, name=f"po{nt}", tag="po",
                                bufs=2)
                    pps = [
                        PS.tile([128, 512], f32, name=f"pp{nt}_{c}",
                                tag="work", bufs=2)
                        for c in range(2)
                    ]
                    for kq in range(KQ):
                        for c in range(2):
                            nc.tensor.matmul(
                                pps[c],
                                lhsT=onormT[kq][:, nt * 128:(nt + 1) * 128],
                                rhs=wo_sb[kq][:, c * 512:(c + 1) * 512],
                                start=(kq == 0),
                                stop=(kq == KQ - 1),
                            )
                    # split the psum evictions across ACT/DVE: in the last
                    # pair's window DVE is near-saturated while ACT has
                    # slack between exps (Copy coexists with the Exp table)
                    nc.scalar.copy(po[:, 0:512], pps[0])
                    nc.vector.tensor_copy(po[:, 512:1024], pps[1])
                    nc.sync.dma_start(out_d[nt * 128:(nt + 1) * 128, :], po)

            for j in range(NPAIR):
                st = stores[j]
                if j == 2:
                    for kq in range(KQ):
                        t = P.tile([128, D], bf16, name=f"wo{kq}", tag=f"wo{kq}")
                        nc.sync.dma_start(t, wo_d[kq * 128:(kq + 1) * 128, :])
                        wo_sb.append(t)
                qT_j, kT_j = st["qT"], st["kT"]
                ve3s = [ve_ring[0][j % 2], ve_ring[1][j % 2]]
                if j + 1 < NPAIR:
                    pe_, pr_ = qkv_emitters(j + 1, stores[j + 1])
                    po_ = sorted(range(len(pe_)), key=lambda i: pr_[i])
                    pend = [pe_[i] for i in po_]
                else:
                    pend = []
                pi = 0
                norm_defer = []
                for qb in range(NQB):
                    b0 = qb * 512
                    opss = [
                        PS.tile([65, 512], f32, name=f"ops{2*j}_{qb}",
                                tag="acc0", bufs=1),
                        PS.tile([65, 512], f32, name=f"ops{2*j+1}_{qb}",
                                tag="acc1", bufs=1),
                    ]
                    for kt in range(NKT):
                        sps = PS.tile(
                            [128, 1024], f32, name=f"s{j}_{qb}_{kt}",
                            tag="spair", bufs=2,
                        )
                        for h2 in range(2):
                            p0 = h2 * 64
                            nc.tensor.matmul(
                                sps[:, h2 * 512:(h2 + 1) * 512],
                                lhsT=kT_j[p0:p0 + 64, kt * 128:(kt + 1) * 128],
                                rhs=qT_j[p0:p0 + 64, b0:b0 + 512],
                                start=True,
                                stop=True,
                                tile_position=(p0, 0),
                            )
                        pt = T.tile(
                            [128, 1024], bf16, name=f"pt{j}_{qb}_{kt}",
                            tag="pt", bufs=10,
                        )
                        nc.scalar.activation(pt, sps, ACTF.Exp, scale=SCALE)
                        for h2 in range(2):
                            nc.tensor.matmul(
                                opss[h2],
                                lhsT=ve3s[h2][:, kt, :],
                                rhs=pt[:, h2 * 512:(h2 + 1) * 512],
                                start=(kt == 0),
                                stop=(kt == NKT - 1),
                            )
                        # interleave next-pair QKV emission across kt slots
                        it = qb * NKT + kt + 1
                        while pi < len(pend) and pi * NKT * NQB < len(pend) * it:
                            pend[pi]()
                            pi += 1
                        # previous block's normalize chain, spread across
                        # this block's kt slots: the 3.3us reciprocals land
                        # between QKV psum evictions in the DVE queue instead
                        # of in front of them, and the pair-3 out-projection
                        # fills PE slack while this block's exps stream
                        if norm_defer:
                            qb_, ocs_, dens_, rlbs_ = norm_defer[0]
                            if kt == 4:
                                norm_recip(j, 0, qb_, dens_, rlbs_)
                            elif kt == 6:
                                norm_recip(j, 1, qb_, dens_, rlbs_)
                            elif kt == 9:
                                norm_muls(j, qb_, ocs_, rlbs_)
                            elif kt == 10:
                                norm_defer.pop(0)
                                if j == NPAIR - 1:
                                    proj_qb(qb_)
                    # evict accumulators to SBUF fast (frees the psum bank);
                    # both denominator rows stack into one tile (partitions
                    # 0 and 64) so one reciprocal later covers both
                    den = T.tile([65, 512], f32, name=f"den{j}_{qb}",
                                 tag="den", bufs=3)
                    ocs = []
                    for h2 in range(2):
                        oc = T.tile([64, 512], f32, name=f"oc{j}_{h2}_{qb}",
                                    tag="oc", bufs=5)
                        nc.vector.tensor_copy(oc, opss[h2][0:64, :])
                        nc.vector.tensor_copy(
                            den[64 * h2:64 * h2 + 1, :], opss[h2][64:65, :]
                        )
                        ocs.append(oc)
                    norm_defer.append((qb, ocs, (den, []), []))
                while pi < len(pend):
                    pend[pi]()
                    pi += 1
                # leftover norm chains (the final block, plus any earlier
                # block still mid-chain)
                while norm_defer:
                    qb_, ocs_, dens_, rlbs_ = norm_defer.pop(0)
                    if len(rlbs_) < 1:
                        norm_recip(j, 0, qb_, dens_, rlbs_)
                    if len(rlbs_) < 2:
                        norm_recip(j, 1, qb_, dens_, rlbs_)
                    norm_muls(j, qb_, ocs_, rlbs_)
                    if j == NPAIR - 1:
                        proj_qb(qb_)

    nc.compile()
    return nc


def _shard_inputs(x, ln_gamma, ln_beta, w_qkv, w_out):
    in_maps = []
    for g in range(2):
        rq = slice(g * INNER_LOC, (g + 1) * INNER_LOC)
        rk = slice(D + g * INNER_LOC, D + (g + 1) * INNER_LOC)
        rv = slice(2 * D + g * INNER_LOC, 2 * D + (g + 1) * INNER_LOC)
        w_slice = np.concatenate(
            [w_qkv[rq], w_qkv[rk], w_qkv[rv]], axis=0
        )                                                           # [1536, 1024]
        w_eff = w_slice * ln_gamma[None, :]
        wqkvT = np.ascontiguousarray(w_eff.T).astype(bfloat16)      # [1024, 1536]
        bias = w_slice.astype(np.float64) @ ln_beta.astype(np.float64)
        bias_2d = np.ascontiguousarray(
            bias.reshape(NOB, 128).T
        ).astype(np.float32)                                        # [128, 12]
        woutT = np.ascontiguousarray(
            w_out[:, g * INNER_LOC:(g + 1) * INNER_LOC].T
        ).astype(bfloat16)                                          # [512, 1024]
        in_maps.append((wqkvT, bias_2d, woutT))

    full = []
    for c in range(NCORES):
        b, g = c // 2, c % 2
        wqkvT, bias_2d, woutT = in_maps[g]
        full.append({
            "x": np.ascontiguousarray(np.asarray(x[b], dtype=np.float32)),
            "wqkvT": wqkvT,
            "qkv_bias": bias_2d,
            "woutT": woutT,
        })
    return full


def kernel(x, ln_gamma, ln_beta, w_qkv, w_out, b_out, _trace=False):
    from concourse import bass_utils

    x = np.asarray(x, dtype=np.float32)
    ln_gamma = np.asarray(ln_gamma, dtype=np.float32)
    ln_beta = np.asarray(ln_beta, dtype=np.float32)
    w_qkv = np.asarray(w_qkv, dtype=np.float32)
    w_out = np.asarray(w_out, dtype=np.float32)
    b_out = np.asarray(b_out, dtype=np.float32)

    if "nc" not in _cache:
        _cache["nc"] = _build()
    nc = _cache["nc"]

    in_maps = _shard_inputs(x, ln_gamma, ln_beta, w_qkv, w_out)
    res = bass_utils.run_bass_kernel_spmd(
        nc, in_maps, core_ids=list(range(NCORES)), trace=_trace
    )
    out = np.empty((B, N, D), dtype=np.float32)
    for b in range(B):
        out[b] = np.asarray(res.results[2 * b]["out"])
        out[b] += np.asarray(res.results[2 * b + 1]["out"])
    out += b_out[None, None, :]
    _cache["last_result"] = res
    return out


# revision 27
# speedup vs baseline: 1.1593x; 1.0238x over previous
"""Trainium2 Bass kernel for a pre-LN multi-head attention block.

Model (per batch b): LayerNorm(x) -> QKV -> 16-head attention (dh=64) ->
output projection + bias.

Sharding over 8 NeuronCores: core c handles batch b = c//2 and head
group g = c%2 (heads 8g..8g+7, all 2048 queries, full 2048 keys).  No
work is duplicated across the pair except the LayerNorm: each core
projects only its own 8 heads' q/k/v and multiplies its 512 inner
features into w_out, producing a partial [2048, 1024] output; the host
sums the two partials per batch (plus b_out).  No collectives.

Device-side layout notes:
 - Activations are kept transposed (feature dim on partitions): every
   matmul contracts over the partition axis.
 - LayerNorm stats run on the vector engine (bn_stats/bn_aggr) and the
   (x-mu)*rstd is one fused tensor_scalar pass, keeping the scalar
   engine free for the softmax exp (the ACT engine is the 2nd-busiest
   resource after the PE array).
 - Scores are computed directly as S^T [nk, nq]; softmax needs no max
   subtraction (scores ~ N(0,1)), so exp is one ScalarE pass and the
   denominator rides along as a ones-column in the PV matmul (M=65).
 - Softmax denominators are reciprocated on the scalar engine ([1,512]
   tiles are single-lane on DVE and cost ~2.3us there vs ~0.6us on ACT).
 - ln_gamma/ln_beta are folded into the QKV weights host-side; b_out is
   added host-side.
 - QKV weights live in SBUF whole (8 tiles of [128,1536] bf16, one DMA
   each); per-pair q/k/v tiles are produced right before that pair's
   attention, interleaved into the previous pair's kt loop so TensorE
   stays densely busy.
 - The output projection for query block qb is emitted right after the
   last head pair's qb normalization, so it overlaps the tail instead
   of serializing after all attention.
"""

import numpy as np
from ml_dtypes import bfloat16

B, N, D = 4, 2048, 1024
HEADS, DH = 16, 64
SCALE = DH ** -0.5
NCORES = 8
HLOC = 8                    # heads per core
INNER_LOC = HLOC * DH       # 512
EPS = 1e-5
NT = N // 128               # 16 sequence tiles (LN)
KD = D // 128               # 8 feature tiles
NKT = N // 128              # 16 key tiles
NPAIR = HLOC // 2           # 4 head pairs per core
NQB = N // 512              # 4 query blocks of 512
NOB = 3 * INNER_LOC // 128  # 12 qkv output row-tiles (q:0-3, k:4-7, v:8-11)
KQ = INNER_LOC // 128       # 4 inner-feature tiles for the out projection

_cache = {}


def _build():
    import concourse.bass as bass
    import concourse.mybir as mybir
    import concourse.bacc as bacc
    import concourse.tile as tile
    from concourse.masks import make_identity

    f32 = mybir.dt.float32
    bf16 = mybir.dt.bfloat16
    AX = mybir.AxisListType
    ALU = mybir.AluOpType
    ACTF = mybir.ActivationFunctionType

    nc = bacc.Bacc(
        "TRN2",
        target_bir_lowering=False,
        debug=False,
        enable_asserts=True,
        num_devices=NCORES,
    )

    x_d = nc.dram_tensor("x", [N, D], f32, kind="ExternalInput").ap()
    wq_d = nc.dram_tensor("wqkvT", [D, 3 * INNER_LOC], bf16,
                          kind="ExternalInput").ap()
    bias_d = nc.dram_tensor("qkv_bias", [128, NOB], f32,
                            kind="ExternalInput").ap()
    wo_d = nc.dram_tensor("woutT", [INNER_LOC, D], bf16,
                          kind="ExternalInput").ap()
    out_d = nc.dram_tensor("out", [N, D], f32, kind="ExternalOutput").ap()

    with tile.TileContext(nc) as tc:
        with (
            tc.tile_pool(name="persist", bufs=1) as P,
            tc.tile_pool(name="ppool", bufs=1, space="PSUM") as PS,
            tc.tile_pool(name="trans", bufs=1) as T,
        ):
            ident = P.tile([128, 128], bf16, name="ident", tag="ident")
            make_identity(nc, ident)
            eps_t = P.tile([128, 1], f32, name="eps_t", tag="eps_t")
            nc.vector.memset(eps_t, EPS)

            bias_sb = P.tile([128, NOB], f32, name="bias_sb", tag="bias_sb")
            nc.sync.dma_start(bias_sb, bias_d)

            # whole QKV weight resident: 8 tiles [128, 1536] bf16
            wq_sb = []
            for k in range(KD):
                t = P.tile([128, 3 * INNER_LOC], bf16, name=f"wq{k}",
                           tag=f"wq{k}")
                nc.sync.dma_start(t, wq_d[k * 128:(k + 1) * 128, :])
                wq_sb.append(t)
            wo_sb = []

            # xnT: transposed normalized activations [d, n] as [128, KD*N]
            xnT = P.tile([128, KD * N], bf16, name="xnT", tag="xnT")
            xnT3 = xnT.rearrange("p (k n) -> p k n", k=KD)
            # normalized attention outputs, transposed: [512 hd, 2048 nq]
            onormT = []
            for p_ in range(KQ):
                onormT.append(
                    P.tile([128, N], bf16, name=f"onormT{p_}", tag=f"onormT{p_}")
                )

            # persistent V_ext ring: [h2][parity] tiles of [nk, 65] blocks;
            # ones column memset once, V part overwritten per pair
            ve_ring = [[None, None], [None, None]]
            for h2 in range(2):
                for par in range(2):
                    ve = P.tile([128, NKT * 65], bf16,
                                name=f"vx{h2}_{par}", tag=f"vx{h2}_{par}")
                    nc.vector.memset(ve, 1.0)
                    ve_ring[h2][par] = ve.rearrange("p (k e) -> p k e", e=65)

            # QKV projection work for pair j is packaged as a list of
            # emission closures so it can be interleaved into pair j-1's
            # attention loop.
            def qkv_emitters(j, store):
                ems = []
                reqs = []
                for key in ("qT", "kT", "vT"):
                    def alloc(j=j, key=key):
                        store[key] = T.tile(
                            [128, N], bf16, name=f"t{key}{j}", tag=key,
                            bufs=3,
                        )
                    ems.append(alloc)
                    reqs.append(-1)
                for c in range(N // 512):
                    for ob, key in ((j, "qT"), (NPAIR + j, "kT"),
                                    (2 * NPAIR + j, "vT")):
                        def chunk(j=j, c=c, ob=ob, key=key):
                            qp = PS.tile(
                                [128, 512], f32, name=f"qp{key}{j}_{c}",
                                tag="work", bufs=2,
                            )
                            for k in range(KD):
                                nc.tensor.matmul(
                                    qp,
                                    lhsT=wq_sb[k][:, ob * 128:(ob + 1) * 128],
                                    rhs=xnT3[:, k, c * 512:(c + 1) * 512],
                                    start=(k == 0),
                                    stop=(k == KD - 1),
                                )
                            dcol = store[key][:, c * 512:(c + 1) * 512]
                            nc.vector.tensor_scalar_add(
                                dcol, qp, bias_sb[:, ob:ob + 1]
                            )
                        ems.append(chunk)
                        reqs.append(4 * c + 3)
                # V_ext for the two heads: [nk, dh | ones] blocks per key tile
                for h2 in range(2):
                    for g2 in range(2):
                        def vtr(j=j, h2=h2, g2=g2):
                            p0 = h2 * 64
                            id64 = ident[p0:p0 + 64, p0:p0 + 64]
                            vT_j = store["vT"]
                            ve3 = ve_ring[h2][j % 2]
                            tp = PS.tile(
                                [128, 512], bf16, name=f"vt{j}_{h2}_{g2}",
                                tag="work", bufs=2,
                            )
                            for i8 in range(8):
                                kt = g2 * 8 + i8
                                nc.tensor.transpose(
                                    tp[:, i8 * 64:(i8 + 1) * 64],
                                    vT_j[p0:p0 + 64, kt * 128:(kt + 1) * 128],
                                    id64,
                                )
                            dest = ve3[:, g2 * 8:(g2 + 1) * 8, 0:64]
                            src = tp.rearrange("p (k e) -> p k e", e=64)
                            nc.vector.tensor_copy(dest, src)
                        ems.append(vtr)
                        reqs.append(8 * g2 + 7)
                return ems, reqs

            stores = [dict() for _ in range(NPAIR)]
            ems0, reqs0 = qkv_emitters(0, stores[0])
            order0 = sorted(range(len(ems0)), key=lambda i: reqs0[i])
            ems0 = [ems0[i] for i in order0]
            reqs0 = [reqs0[i] for i in order0]
            e0i = 0

            # ---- Phase A: LayerNorm + transpose, pipelined over seq tiles.
            # Stats on DVE (bn_stats/bn_aggr), normalize is one fused
            # tensor_scalar pass; ACT only does the tiny rsqrt.
            for nt in range(NT):
                x_t = T.tile([128, D], f32, name=f"x{nt}", tag="x", bufs=3)
                nc.sync.dma_start(x_t, x_d[nt * 128:(nt + 1) * 128, :])
                x3 = x_t.rearrange("p (c f) -> p c f", c=2)
                st6 = T.tile([128, 2, 6], f32, name=f"st{nt}", tag="st", bufs=3)
                nc.vector.bn_stats(st6[:, 0, :], x3[:, 0, :])
                nc.vector.bn_stats(st6[:, 1, :], x3[:, 1, :])
                mv = T.tile([128, 2], f32, name=f"mv{nt}", tag="mv", bufs=3)
                nc.vector.bn_aggr(mv, st6)
                std = T.tile([128, 1], f32, name=f"sd{nt}", tag="sd", bufs=3)
                nc.scalar.activation(std, mv[:, 1:2], ACTF.Sqrt, bias=eps_t)
                rstd = T.tile([128, 1], f32, name=f"rs{nt}", tag="rs", bufs=3)
                nc.vector.reciprocal(rstd, std)
                xhat = T.tile([128, D], bf16, name=f"xh{nt}", tag="xh", bufs=3)
                nc.vector.tensor_scalar(
                    xhat, x_t, mv[:, 0:1], rstd, ALU.subtract, ALU.mult
                )
                for g2 in range(2):
                    tp = PS.tile(
                        [128, 512], bf16, name=f"tp{nt}_{g2}",
                        tag="work", bufs=2
                    )
                    for j in range(4):
                        kd = g2 * 4 + j
                        nc.tensor.transpose(
                            tp[:, j * 128:(j + 1) * 128],
                            xhat[:, kd * 128:(kd + 1) * 128],
                            ident,
                        )
                    dest = xnT3[:, g2 * 4:(g2 + 1) * 4, nt * 128:(nt + 1) * 128]
                    src = tp.rearrange("p (k n) -> p k n", k=4)
                    # alternate the psum evictions between the two engines
                    # that can read PSUM, to balance phase-A load
                    if (nt + g2) % 2 == 0:
                        nc.vector.tensor_copy(dest, src)
                    else:
                        nc.scalar.copy(dest, src)
                # interleave pair-0 QKV emission once its LN inputs
                # have been emitted (Tile deps follow emission order)
                while e0i < len(ems0) and reqs0[e0i] <= nt:
                    ems0[e0i]()
                    e0i += 1

            # ---- Phases B+C: per head pair, attention row-packed via
            # tile_position so K stays covered.
            while e0i < len(ems0):
                ems0[e0i]()
                e0i += 1

            def norm_recip(p_, h2, qb, dens, rlbs):
                # both heads' denominators are stacked at partitions 0/64 of
                # one tile, so h2==0 pays a single 3.3us DVE reciprocal for
                # both (reciprocal cost is free-size driven).  h2==1 only
                # rehomes its row to partition 0 — partition_broadcast
                # silently reads partition 0 of its source (HW-probed).
                den, rls = dens
                if h2 == 0:
                    rl = T.tile([65, 512], f32, name=f"rl{p_}_{qb}",
                                tag="rl", bufs=3)
                    nc.vector.reciprocal(rl, den)
                    rls.append(rl)
                    src = rl[0:1, :]
                else:
                    rl1 = T.tile([1, 512], f32, name=f"rl1{p_}_{qb}",
                                 tag="rl1", bufs=3)
                    nc.vector.tensor_copy(rl1, rls[0][64:65, :])
                    src = rl1
                rlb = T.tile([64, 512], f32, name=f"rlb{p_}_{h2}_{qb}",
                             tag="rlb", bufs=3)
                nc.gpsimd.partition_broadcast(rlb, src, channels=64)
                rlbs.append(rlb)

            def norm_muls(p_, qb, ocs, rlbs):
                b0 = qb * 512
                for h2 in range(2):
                    p0 = h2 * 64
                    nc.vector.tensor_mul(
                        onormT[p_][p0:p0 + 64, b0:b0 + 512],
                        ocs[h2],
                        rlbs[h2],
                    )

            def proj_qb(qb):
                # out rows qb*512 .. qb*512+512, all 1024 cols; the two
                # 512-col accumulation chains are interleaved (2 PE streams)
                for nt in range(4 * qb, 4 * qb + 4):
                    po = T.tile([128, D], f32, name=f"po{nt}", tag="po",
                                bufs=2)
                    pps = [
                        PS.tile([128, 512], f32, name=f"pp{nt}_{c}",
                                tag="work", bufs=2)
                        for c in range(2)
                    ]
                    for kq in range(KQ):
                        for c in range(2):
                            nc.tensor.matmul(
                                pps[c],
                                lhsT=onormT[kq][:, nt * 128:(nt + 1) * 128],
                                rhs=wo_sb[kq][:, c * 512:(c + 1) * 512],
                                start=(kq == 0),
                                stop=(kq == KQ - 1),
                            )
                    # split the psum evictions across ACT/DVE: in the last
                    # pair's window DVE is near-saturated while ACT has
                    # slack between exps (Copy coexists with the Exp table)
                    nc.scalar.copy(po[:, 0:512], pps[0])
                    nc.vector.tensor_copy(po[:, 512:1024], pps[1])
                    nc.sync.dma_start(out_d[nt * 128:(nt + 1) * 128, :], po)

            for j in range(NPAIR):
                st = stores[j]
                if j == 2:
                    for kq in range(KQ):
                        t = P.tile([128, D], bf16, name=f"wo{kq}", tag=f"wo{kq}")
                        nc.sync.dma_start(t, wo_d[kq * 128:(kq + 1) * 128, :])
                        wo_sb.append(t)
                qT_j, kT_j = st["qT"], st["kT"]
                ve3s = [ve_ring[0][j % 2], ve_ring[1][j % 2]]
                if j + 1 < NPAIR:
                    pe_, pr_ = qkv_emitters(j + 1, stores[j + 1])
                    po_ = sorted(range(len(pe_)), key=lambda i: pr_[i])
                    pend = [pe_[i] for i in po_]
                else:
                    pend = []
                pi = 0
                norm_defer = []
                for qb in range(NQB):
                    b0 = qb * 512
                    opss = [
                        PS.tile([65, 512], f32, name=f"ops{2*j}_{qb}",
                                tag="acc0", bufs=1),
                        PS.tile([65, 512], f32, name=f"ops{2*j+1}_{qb}",
                                tag="acc1", bufs=1),
                    ]
                    for kt in range(NKT):
                        sps = PS.tile(
                            [128, 1024], f32, name=f"s{j}_{qb}_{kt}",
                            tag="spair", bufs=2,
                        )
                        for h2 in range(2):
                            p0 = h2 * 64
                            nc.tensor.matmul(
                                sps[:, h2 * 512:(h2 + 1) * 512],
                                lhsT=kT_j[p0:p0 + 64, kt * 128:(kt + 1) * 128],
                                rhs=qT_j[p0:p0 + 64, b0:b0 + 512],
                                start=True,
                                stop=True,
                                tile_position=(p0, 0),
                            )
                        pt = T.tile(
                            [128, 1024], bf16, name=f"pt{j}_{qb}_{kt}",
                            tag="pt", bufs=10,
                        )
                        nc.scalar.activation(pt, sps, ACTF.Exp, scale=SCALE)
                        for h2 in range(2):
                            nc.tensor.matmul(
                                opss[h2],
                                lhsT=ve3s[h2][:, kt, :],
                                rhs=pt[:, h2 * 512:(h2 + 1) * 512],
                                start=(kt == 0),
                                stop=(kt == NKT - 1),
                            )
                        # interleave next-pair QKV emission across kt slots
                        it = qb * NKT + kt + 1
                        while pi < len(pend) and pi * NKT * NQB < len(pend) * it:
                            pend[pi]()
                            pi += 1
                        # previous block's normalize chain, spread across
                        # this block's kt slots: the 3.3us reciprocals land
                        # between QKV psum evictions in the DVE queue instead
                        # of in front of them, and the pair-3 out-projection
                        # fills PE slack while this block's exps stream
                        if norm_defer:
                            qb_, ocs_, dens_, rlbs_ = norm_defer[0]
                            if kt == 4:
                                norm_recip(j, 0, qb_, dens_, rlbs_)
                            elif kt == 6:
                                norm_recip(j, 1, qb_, dens_, rlbs_)
                            elif kt == 9:
                                norm_muls(j, qb_, ocs_, rlbs_)
                            elif kt == 10:
                                norm_defer.pop(0)
                                if j == NPAIR - 1:
                                    proj_qb(qb_)
                    # evict accumulators to SBUF fast (frees the psum bank);
                    # both denominator rows stack into one tile (partitions
                    # 0 and 64) so one reciprocal later covers both
                    den = T.tile([65, 512], f32, name=f"den{j}_{qb}",
                                 tag="den", bufs=3)
                    ocs = []
                    for h2 in range(2):
                        oc = T.tile([64, 512], f32, name=f"oc{j}_{h2}_{qb}",
                                    tag="oc", bufs=5)
                        nc.vector.tensor_copy(oc, opss[h2][0:64, :])
                        nc.vector.tensor_copy(
                            den[64 * h2:64 * h2 + 1, :], opss[h2][64:65, :]
                        )
                        ocs.append(oc)
                    norm_defer.append((qb, ocs, (den, []), []))
                while pi < len(pend):
                    pend[pi]()
                    pi += 1
                # leftover norm chains (the final block, plus any earlier
                # block still mid-chain)
                while norm_defer:
                    qb_, ocs_, dens_, rlbs_ = norm_defer.pop(0)
                    if len(rlbs_) < 1:
                        norm_recip(j, 0, qb_, dens_, rlbs_)
                    if len(rlbs_) < 2:
                        norm_recip(j, 1, qb_, dens_, rlbs_)
                    norm_muls(j, qb_, ocs_, rlbs_)
                    if j == NPAIR - 1:
                        proj_qb(qb_)

    nc.compile()
    return nc


def _shard_inputs(x, ln_gamma, ln_beta, w_qkv, w_out):
    in_maps = []
    for g in range(2):
        rq = slice(g * INNER_LOC, (g + 1) * INNER_LOC)
        rk = slice(D + g * INNER_LOC, D + (g + 1) * INNER_LOC)
        rv = slice(2 * D + g * INNER_LOC, 2 * D + (g + 1) * INNER_LOC)
        w_slice = np.concatenate(
            [w_qkv[rq], w_qkv[rk], w_qkv[rv]], axis=0
        )                                                           # [1536, 1024]
        w_eff = w_slice * ln_gamma[None, :]
        wqkvT = np.ascontiguousarray(w_eff.T).astype(bfloat16)      # [1024, 1536]
        bias = w_slice.astype(np.float64) @ ln_beta.astype(np.float64)
        bias_2d = np.ascontiguousarray(
            bias.reshape(NOB, 128).T
        ).astype(np.float32)                                        # [128, 12]
        woutT = np.ascontiguousarray(
            w_out[:, g * INNER_LOC:(g + 1) * INNER_LOC].T
        ).astype(bfloat16)                                          # [512, 1024]
        in_maps.append((wqkvT, bias_2d, woutT))

    full = []
    for c in range(NCORES):
        b, g = c // 2, c % 2
        wqkvT, bias_2d, woutT = in_maps[g]
        full.append({
            "x": np.ascontiguousarray(np.asarray(x[b], dtype=np.float32)),
            "wqkvT": wqkvT,
            "qkv_bias": bias_2d,
            "woutT": woutT,
        })
    return full


def kernel(x, ln_gamma, ln_beta, w_qkv, w_out, b_out, _trace=False):
    from concourse import bass_utils

    x = np.asarray(x, dtype=np.float32)
    ln_gamma = np.asarray(ln_gamma, dtype=np.float32)
    ln_beta = np.asarray(ln_beta, dtype=np.float32)
    w_qkv = np.asarray(w_qkv, dtype=np.float32)
    w_out = np.asarray(w_out, dtype=np.float32)
    b_out = np.asarray(b_out, dtype=np.float32)

    if "nc" not in _cache:
        _cache["nc"] = _build()
    nc = _cache["nc"]

    in_maps = _shard_inputs(x, ln_gamma, ln_beta, w_qkv, w_out)
    res = bass_utils.run_bass_kernel_spmd(
        nc, in_maps, core_ids=list(range(NCORES)), trace=_trace
    )
    out = np.empty((B, N, D), dtype=np.float32)
    for b in range(B):
        out[b] = np.asarray(res.results[2 * b]["out"])
        out[b] += np.asarray(res.results[2 * b + 1]["out"])
    out += b_out[None, None, :]
    _cache["last_result"] = res
    return out
